# revision 2
# baseline (speedup 1.0000x reference)
"""Trainium2 Bass kernel for InvariantMessagePassingTP (v2: int8 payload).

out[n, lm, c] = sum_{e: recv[e]=n} edge_attrs[e,lm] * tp_weights[e,l(lm),c]
                * node_feats[recv[e], c]

Key algebra: the gather index equals the scatter index, so node_feats
factors out of the segment sum:
    out[n] = node_feats[n] * G[n],  G[n,lm,c] = sum_e A[e,lm] * W[e,l,c]
The device computes G via the one-hot matmul trick; the host applies
node_feats at the end.

Quantization (all scales cancel via host-side folding):
  W[e,l,c]  -> int8 Wq with per-(e,l) scale s_W; s_W is folded into A.
  At[e,(lm,k)] = A'[e,lm]*onehot_k (A' = A*s_W) -> int8 Atq with per-
  PSUM-row scale s_col[t,lm,k], folded into the host output gather.

Device per tile (128 edges, <=8 nodes):
  u|at = cast(int8 payload [256 W | 128 At]) -> bf16     (DVE, 2x)
  phase A: ps[0:96, :]   = At[:,0:96]^T  @ u[:,128:256]  (PE: l2,l3)
  phase B: ps[96:128, :] = At[:,96:128]^T @ u[:,0:128]   (PE: l0,l1)
  extraction (ACT) -> bf16 stage -> 4-fragment DMA to slots[row,t,c]
Host: slots * s_col -> scatter-add per tile -> * node_feats.
"""

import sys

sys.path.insert(0, "/opt/trn_rl_repo")

import numpy as np
import ml_dtypes

import concourse.bass as bass
import concourse.bacc as bacc
import concourse.tile as tile
from concourse import mybir
from concourse.bass_utils import run_bass_kernel_spmd

NPBF = ml_dtypes.bfloat16
BF16 = mybir.dt.bfloat16
I8 = mybir.dt.int8
F32 = mybir.dt.float32

NNODES = 25000
NEDGES = 400000
NCHAN = 64
N_CORES = 8
NPC = NNODES // N_CORES        # nodes per core
TB = 384                       # int8 bytes per tile per partition
CHUNK = 32                     # tiles per chunk (stage + out-DMA granularity)
HC = 16                        # tiles per in-DMA + cast op
PSB = 8                        # tiles per PSUM batch

L_OF_LM = np.array([0, 1, 1, 1, 2, 2, 2, 2, 2, 3, 3, 3, 3, 3, 3, 3], np.int64)
# row-block order of lm in At / PSUM / slots: l2,l3 first (96 rows), then
# l0,l1 (32 rows) - phase A covers rows 0:96, phase B rows 96:128.
PERM_LM = np.array([4, 5, 6, 7, 8, 9, 10, 11, 12, 13, 14, 15, 0, 1, 2, 3])
L_OF_PB = L_OF_LM[PERM_LM]     # l value per perm-block index

_PROGRAM_CACHE = {}


def _greedy_groups(deg, node0):
    """Group consecutive nodes: <=8 nodes, <=128 edges per group."""
    groups = []
    n = len(deg)
    i = 0
    while i < n:
        if deg[i] > 128:
            rem = deg[i]
            while rem > 0:
                take = min(128, rem)
                groups.append((node0 + i, 1, take))
                rem -= take
            i += 1
            continue
        cnt = 0
        edges = 0
        while i + cnt < n and cnt < 8 and edges + deg[i + cnt] <= 128:
            edges += deg[i + cnt]
            cnt += 1
        groups.append((node0 + i, cnt, edges))
        i += cnt
    return groups


def _build_schedule(receiver_list):
    recv = np.asarray(receiver_list).astype(np.int64)
    deg = np.bincount(recv, minlength=NNODES)
    per_core = []
    for c in range(N_CORES):
        per_core.append(_greedy_groups(deg[c * NPC:(c + 1) * NPC], c * NPC))
    t_max = max(len(g) for g in per_core)
    t_u = -(-t_max // CHUNK) * CHUNK  # round up to whole chunks
    return recv, deg, per_core, t_u


def _pack_inputs(edge_attrs, tp_weights, recv, per_core, t_u):
    a_f = np.asarray(edge_attrs, np.float32)
    w_f = np.asarray(tp_weights, np.float32).reshape(NEDGES, 4, NCHAN)
    node_e0 = np.searchsorted(recv, np.arange(NNODES + 1))

    # --- global edge quantization (vectorized over all edges) ---
    s_w = np.abs(w_f).max(axis=2) / 127.0            # [E, 4]
    s_w = np.maximum(s_w, 1e-30)
    wq = np.rint(w_f / s_w[:, :, None]).astype(np.int8).reshape(NEDGES, 256)
    a_p = a_f[:, PERM_LM] * s_w[:, L_OF_PB]          # A' [E, 16] perm order

    in_maps = []
    slot_maps = []   # per core: (node_start, n_nodes) per tile
    scale_maps = []  # per core: s_col [T, 16, 8] fp32
    for c in range(N_CORES):
        groups = per_core[c]
        T = t_u
        # per-edge tile assignment for this core
        n_e = sum(g[2] for g in groups)
        pos = np.empty(n_e, np.int64)     # row in [T*128] canvas
        tk = np.empty(n_e, np.int64)      # t*8 + loc (column-group id)
        eidx = np.empty(n_e, np.int64)    # global edge index
        smap = []
        e_cursor = {}
        o = 0
        for t, (n0, k, ne) in enumerate(groups):
            smap.append((n0, k))
            if ne == 0:
                continue
            e0 = node_e0[n0] + e_cursor.get(n0, 0) if k == 1 else node_e0[n0]
            if k == 1:
                e_cursor[n0] = e_cursor.get(n0, 0) + ne
            sl = slice(o, o + ne)
            eidx[sl] = np.arange(e0, e0 + ne)
            pos[sl] = t * 128 + np.arange(ne)
            loc = (recv[e0:e0 + ne] - n0) if k > 1 else np.zeros(ne, np.int64)
            tk[sl] = t * 8 + loc
            o += ne
        while len(smap) < T:
            smap.append((0, 0))

        # per-column scales s_col[t*8+loc, pb]
        a_core = a_p[eidx]                              # [n_e, 16]
        s_col = np.zeros((T * 8, 16), np.float32)
        np.maximum.at(s_col, tk, np.abs(a_core))
        s_col = np.maximum(s_col / 127.0, 1e-30)
        atq_vals = np.rint(a_core / s_col[tk]).astype(np.int8)   # [n_e, 16]

        X = np.zeros((T * 128, TB), np.int8)
        X[pos, 0:256] = wq[eidx]
        at_cols = 256 + (np.arange(16) * 8)[None, :] + (tk % 8)[:, None]
        X[pos[:, None], at_cols] = atq_vals

        buf = X.reshape(T, 128, TB).transpose(1, 0, 2).reshape(128, T * TB)
        in_maps.append({"inp": np.ascontiguousarray(buf)})
        slot_maps.append(smap)
        scale_maps.append(s_col.reshape(T, 8, 16).transpose(0, 2, 1))
    return in_maps, slot_maps, scale_maps


def _build_program(t_u):
    nc = bacc.Bacc("TRN2", target_bir_lowering=False, debug=False,
                   num_devices=N_CORES)
    T = t_u
    in_d = nc.dram_tensor("inp", [128, T * TB], I8, kind="ExternalInput").ap()
    # slots[row = perm-lm-block*8 + k, tile, c]
    out_d = nc.dram_tensor("out", [128, T, 64], BF16,
                           kind="ExternalOutput").ap()

    n_chunks = T // CHUNK
    with tile.TileContext(nc) as tc:
        with tc.tile_pool(name="ld", bufs=6) as ld_pool, \
             tc.tile_pool(name="u", bufs=4) as u_pool, \
             tc.tile_pool(name="st", bufs=3) as st_pool, \
             tc.tile_pool(name="ps", bufs=4, space="PSUM") as ps_pool:
            for ch in range(n_chunks):
                t0 = ch * CHUNK
                stage = st_pool.tile([128, 2, CHUNK, 64], BF16, tag="stage")
                for h0 in range(0, CHUNK, HC):
                    th = t0 + h0
                    ld = ld_pool.tile([128, HC * TB], I8, tag="ld")
                    nc.sync.dma_start(
                        out=ld,
                        in_=bass.AP(
                            tensor=in_d.tensor, offset=th * TB,
                            ap=[[T * TB, 128], [1, HC * TB]]),
                    )
                    u = u_pool.tile([128, HC, TB], BF16, tag="u")
                    nc.vector.tensor_copy(
                        u, ld.rearrange("p (t b) -> p t b", b=TB))
                    for p0 in range(0, HC, PSB):
                        ps = ps_pool.tile([128, PSB, 128], F32, tag="ps")
                        for k in range(PSB):
                            t = p0 + k
                            nc.tensor.matmul(
                                ps[0:96, k], u[:, t, 256:352],
                                u[:, t, 128:256],
                                start=True, stop=True)
                            nc.tensor.matmul(
                                ps[96:128, k], u[:, t, 352:384],
                                u[:, t, 0:128],
                                start=True, stop=True,
                                tile_position=(0, 96))
                        nc.scalar.copy(
                            bass.AP(
                                tensor=stage.tensor,
                                offset=stage.offset + (h0 + p0) * 64,
                                ap=[stage.ap[0], [64, PSB], [CHUNK * 64, 2],
                                    [1, 64]]),
                            ps,
                        )
                # 4 out-DMA fragments per chunk; DMA picks valid rows
                for (r0, r1, half) in ((0, 40, 0), (40, 96, 1),
                                       (96, 104, 0), (104, 128, 1)):
                    nc.sync.dma_start(
                        out=bass.AP(
                            tensor=out_d.tensor,
                            offset=r0 * (T * 64) + t0 * 64,
                            ap=[[T * 64, r1 - r0], [64, CHUNK], [1, 64]]),
                        in_=stage[r0:r1, half],
                    )
    nc.compile()
    return nc


def kernel(node_feats, edge_attrs, tp_weights, receiver_list, nnodes,
           _trace=False):
    node_feats = np.asarray(node_feats)
    edge_attrs = np.asarray(edge_attrs)
    tp_weights = np.asarray(tp_weights)
    receiver_list = np.asarray(receiver_list)
    nnodes = int(nnodes)
    assert node_feats.shape == (NNODES, NCHAN) and nnodes == NNODES
    assert tp_weights.shape == (NEDGES, 4, NCHAN)

    recv, deg, per_core, t_u = _build_schedule(receiver_list)
    key = int(t_u)
    if key not in _PROGRAM_CACHE:
        _PROGRAM_CACHE[key] = _build_program(t_u)
    nc = _PROGRAM_CACHE[key]

    in_maps, slot_maps, scale_maps = _pack_inputs(
        edge_attrs, tp_weights, recv, per_core, t_u)
    res = run_bass_kernel_spmd(nc, in_maps, list(range(N_CORES)),
                               trace=_trace)

    inv = np.argsort(PERM_LM)  # lm -> row-block index
    g = np.zeros((NNODES, 16, NCHAN), np.float32)
    for c in range(N_CORES):
        slots = res.results[c]["out"].astype(np.float32)  # [128, T, 64]
        slots = slots.reshape(16, 8, -1, NCHAN)           # [pb, k, T, c]
        slots *= scale_maps[c].transpose(1, 2, 0)[:, :, :, None]
        slots = slots[inv]                                # [lm, k, T, c]
        smap = slot_maps[c]
        for t, (n0, k) in enumerate(smap):
            if k == 0:
                continue
            g[n0:n0 + k] += slots[:, 0:k, t, :].transpose(1, 0, 2)
    out = g * node_feats.astype(np.float32)[:, None, :]
    if _trace:
        return out, res
    return out


# revision 3
# speedup vs baseline: 1.7673x; 1.7673x over previous
"""Trainium2 Bass kernel for InvariantMessagePassingTP (v2: int8 payload).

out[n, lm, c] = sum_{e: recv[e]=n} edge_attrs[e,lm] * tp_weights[e,l(lm),c]
                * node_feats[recv[e], c]

Key algebra: the gather index equals the scatter index, so node_feats
factors out of the segment sum:
    out[n] = node_feats[n] * G[n],  G[n,lm,c] = sum_e A[e,lm] * W[e,l,c]
The device computes G via the one-hot matmul trick; the host applies
node_feats at the end.

Quantization (all scales cancel via host-side folding):
  W[e,l,c]  -> int8 Wq with per-(e,l) scale s_W; s_W is folded into A.
  At[e,(lm,k)] = A'[e,lm]*onehot_k (A' = A*s_W) -> int8 Atq with per-
  PSUM-row scale s_col[t,lm,k], folded into the host output gather.

Device per tile (128 edges, <=8 nodes):
  u|at = cast(int8 payload [256 W | 128 At]) -> bf16     (DVE, 2x)
  phase A: ps[0:96, :]   = At[:,0:96]^T  @ u[:,128:256]  (PE: l2,l3)
  phase B: ps[96:128, :] = At[:,96:128]^T @ u[:,0:128]   (PE: l0,l1)
  extraction (ACT) -> bf16 stage -> 4-fragment DMA to slots[row,t,c]
Host: slots * s_col -> scatter-add per tile -> * node_feats.
"""

import sys

sys.path.insert(0, "/opt/trn_rl_repo")

import numpy as np
import ml_dtypes

import concourse.bass as bass
import concourse.bacc as bacc
import concourse.tile as tile
from concourse import mybir
from concourse.bass_utils import run_bass_kernel_spmd

NPBF = ml_dtypes.bfloat16
BF16 = mybir.dt.bfloat16
I8 = mybir.dt.int8
F32 = mybir.dt.float32

NNODES = 25000
NEDGES = 400000
NCHAN = 64
N_CORES = 8
NPC = NNODES // N_CORES        # nodes per core
TB = 384                       # int8 bytes per tile per partition
CHUNK = 32                     # tiles per chunk (stage + out-DMA granularity)
HC = 16                        # tiles per in-DMA + cast op
PSB = 8                        # tiles per PSUM batch

L_OF_LM = np.array([0, 1, 1, 1, 2, 2, 2, 2, 2, 3, 3, 3, 3, 3, 3, 3], np.int64)
# row-block order of lm in At / PSUM / slots: l2,l3 first (96 rows), then
# l0,l1 (32 rows) - phase A covers rows 0:96, phase B rows 96:128.
PERM_LM = np.array([4, 5, 6, 7, 8, 9, 10, 11, 12, 13, 14, 15, 0, 1, 2, 3])
L_OF_PB = L_OF_LM[PERM_LM]     # l value per perm-block index

_PROGRAM_CACHE = {}


def _greedy_groups(deg, node0):
    """Group consecutive nodes: <=8 nodes, <=128 edges per group."""
    groups = []
    n = len(deg)
    i = 0
    while i < n:
        if deg[i] > 128:
            rem = deg[i]
            while rem > 0:
                take = min(128, rem)
                groups.append((node0 + i, 1, take))
                rem -= take
            i += 1
            continue
        cnt = 0
        edges = 0
        while i + cnt < n and cnt < 8 and edges + deg[i + cnt] <= 128:
            edges += deg[i + cnt]
            cnt += 1
        groups.append((node0 + i, cnt, edges))
        i += cnt
    return groups


def _build_schedule(receiver_list):
    recv = np.asarray(receiver_list).astype(np.int64)
    deg = np.bincount(recv, minlength=NNODES)
    per_core = []
    for c in range(N_CORES):
        per_core.append(_greedy_groups(deg[c * NPC:(c + 1) * NPC], c * NPC))
    t_max = max(len(g) for g in per_core)
    t_u = -(-t_max // CHUNK) * CHUNK  # round up to whole chunks
    return recv, deg, per_core, t_u


def _pack_inputs(edge_attrs, tp_weights, recv, per_core, t_u):
    a_f = np.asarray(edge_attrs, np.float32)
    w_f = np.asarray(tp_weights, np.float32).reshape(NEDGES, 4, NCHAN)
    node_e0 = np.searchsorted(recv, np.arange(NNODES + 1))

    # --- global edge quantization (vectorized over all edges) ---
    s_w = np.abs(w_f).max(axis=2) / 127.0            # [E, 4]
    s_w = np.maximum(s_w, 1e-30)
    wq = np.rint(w_f / s_w[:, :, None]).astype(np.int8).reshape(NEDGES, 256)
    a_p = a_f[:, PERM_LM] * s_w[:, L_OF_PB]          # A' [E, 16] perm order

    in_maps = []
    slot_maps = []   # per core: (node_start, n_nodes) per tile
    scale_maps = []  # per core: s_col [T, 16, 8] fp32
    for c in range(N_CORES):
        groups = per_core[c]
        T = t_u
        # per-edge tile assignment for this core
        n_e = sum(g[2] for g in groups)
        pos = np.empty(n_e, np.int64)     # row in [T*128] canvas
        tk = np.empty(n_e, np.int64)      # t*8 + loc (column-group id)
        eidx = np.empty(n_e, np.int64)    # global edge index
        smap = []
        e_cursor = {}
        o = 0
        for t, (n0, k, ne) in enumerate(groups):
            smap.append((n0, k))
            if ne == 0:
                continue
            e0 = node_e0[n0] + e_cursor.get(n0, 0) if k == 1 else node_e0[n0]
            if k == 1:
                e_cursor[n0] = e_cursor.get(n0, 0) + ne
            sl = slice(o, o + ne)
            eidx[sl] = np.arange(e0, e0 + ne)
            pos[sl] = t * 128 + np.arange(ne)
            loc = (recv[e0:e0 + ne] - n0) if k > 1 else np.zeros(ne, np.int64)
            tk[sl] = t * 8 + loc
            o += ne
        while len(smap) < T:
            smap.append((0, 0))

        # per-column scales s_col[t*8+loc, pb]
        a_core = a_p[eidx]                              # [n_e, 16]
        s_col = np.zeros((T * 8, 16), np.float32)
        np.maximum.at(s_col, tk, np.abs(a_core))
        s_col = np.maximum(s_col / 127.0, 1e-30)
        atq_vals = np.rint(a_core / s_col[tk]).astype(np.int8)   # [n_e, 16]

        X = np.zeros((T * 128, TB), np.int8)
        X[pos, 0:256] = wq[eidx]
        at_cols = 256 + (np.arange(16) * 8)[None, :] + (tk % 8)[:, None]
        X[pos[:, None], at_cols] = atq_vals

        buf = X.reshape(T, 128, TB).transpose(1, 0, 2).reshape(128, T * TB)
        in_maps.append({"inp": np.ascontiguousarray(buf)})
        slot_maps.append(smap)
        scale_maps.append(s_col.reshape(T, 8, 16).transpose(0, 2, 1))
    return in_maps, slot_maps, scale_maps


def _build_program(t_u):
    nc = bacc.Bacc("TRN2", target_bir_lowering=False, debug=False,
                   num_devices=N_CORES)
    T = t_u
    in_d = nc.dram_tensor("inp", [128, T * TB], I8, kind="ExternalInput").ap()
    # slots[row = perm-lm-block*8 + k, tile, c]
    out_d = nc.dram_tensor("out", [128, T, 64], BF16,
                           kind="ExternalOutput").ap()

    n_chunks = T // CHUNK
    with tile.TileContext(nc) as tc:
        with tc.tile_pool(name="ld", bufs=8) as ld_pool, \
             tc.tile_pool(name="u", bufs=6) as u_pool, \
             tc.tile_pool(name="st", bufs=4) as st_pool, \
             tc.tile_pool(name="ps", bufs=4, space="PSUM") as ps_pool:
            for ch in range(n_chunks):
                t0 = ch * CHUNK
                stage = st_pool.tile([128, 2, CHUNK, 64], BF16, tag="stage")
                for h0 in range(0, CHUNK, HC):
                    th = t0 + h0
                    ld = ld_pool.tile([128, HC * TB], I8, tag="ld")
                    nc.sync.dma_start(
                        out=ld,
                        in_=bass.AP(
                            tensor=in_d.tensor, offset=th * TB,
                            ap=[[T * TB, 128], [1, HC * TB]]),
                    )
                    u = u_pool.tile([128, HC, TB], BF16, tag="u")
                    nc.vector.tensor_copy(
                        u, ld.rearrange("p (t b) -> p t b", b=TB))
                    for p0 in range(0, HC, PSB):
                        ps = ps_pool.tile([128, PSB, 128], F32, tag="ps")
                        for k in range(PSB):
                            t = p0 + k
                            nc.tensor.matmul(
                                ps[0:96, k], u[:, t, 256:352],
                                u[:, t, 128:256],
                                start=True, stop=True)
                        for k in range(PSB):
                            t = p0 + k
                            nc.tensor.matmul(
                                ps[96:128, k], u[:, t, 352:384],
                                u[:, t, 0:128],
                                start=True, stop=True,
                                tile_position=(0, 96))
                        nc.scalar.copy(
                            bass.AP(
                                tensor=stage.tensor,
                                offset=stage.offset + (h0 + p0) * 64,
                                ap=[stage.ap[0], [64, PSB], [CHUNK * 64, 2],
                                    [1, 64]]),
                            ps,
                        )
                # 4 out-DMA fragments per chunk; DMA picks valid rows
                for (r0, r1, half) in ((0, 40, 0), (40, 96, 1),
                                       (96, 104, 0), (104, 128, 1)):
                    nc.sync.dma_start(
                        out=bass.AP(
                            tensor=out_d.tensor,
                            offset=r0 * (T * 64) + t0 * 64,
                            ap=[[T * 64, r1 - r0], [64, CHUNK], [1, 64]]),
                        in_=stage[r0:r1, half],
                    )
    nc.compile()
    return nc


def kernel(node_feats, edge_attrs, tp_weights, receiver_list, nnodes,
           _trace=False):
    node_feats = np.asarray(node_feats)
    edge_attrs = np.asarray(edge_attrs)
    tp_weights = np.asarray(tp_weights)
    receiver_list = np.asarray(receiver_list)
    nnodes = int(nnodes)
    assert node_feats.shape == (NNODES, NCHAN) and nnodes == NNODES
    assert tp_weights.shape == (NEDGES, 4, NCHAN)

    recv, deg, per_core, t_u = _build_schedule(receiver_list)
    key = int(t_u)
    if key not in _PROGRAM_CACHE:
        _PROGRAM_CACHE[key] = _build_program(t_u)
    nc = _PROGRAM_CACHE[key]

    in_maps, slot_maps, scale_maps = _pack_inputs(
        edge_attrs, tp_weights, recv, per_core, t_u)
    res = run_bass_kernel_spmd(nc, in_maps, list(range(N_CORES)),
                               trace=_trace)

    inv = np.argsort(PERM_LM)  # lm -> row-block index
    g = np.zeros((NNODES, 16, NCHAN), np.float32)
    for c in range(N_CORES):
        slots = res.results[c]["out"].astype(np.float32)  # [128, T, 64]
        slots = slots.reshape(16, 8, -1, NCHAN)           # [pb, k, T, c]
        slots *= scale_maps[c].transpose(1, 2, 0)[:, :, :, None]
        slots = slots[inv]                                # [lm, k, T, c]
        smap = slot_maps[c]
        for t, (n0, k) in enumerate(smap):
            if k == 0:
                continue
            g[n0:n0 + k] += slots[:, 0:k, t, :].transpose(1, 0, 2)
    out = g * node_feats.astype(np.float32)[:, None, :]
    if _trace:
        return out, res
    return out


# revision 7
# speedup vs baseline: 1.9828x; 1.1219x over previous
"""Trainium2 Bass kernel for InvariantMessagePassingTP (v2: int8 payload).

out[n, lm, c] = sum_{e: recv[e]=n} edge_attrs[e,lm] * tp_weights[e,l(lm),c]
                * node_feats[recv[e], c]

Key algebra: the gather index equals the scatter index, so node_feats
factors out of the segment sum:
    out[n] = node_feats[n] * G[n],  G[n,lm,c] = sum_e A[e,lm] * W[e,l,c]
The device computes G via the one-hot matmul trick; the host applies
node_feats at the end.

Quantization (all scales cancel via host-side folding):
  W[e,l,c]  -> int8 Wq with per-(e,l) scale s_W; s_W is folded into A.
  At[e,(lm,k)] = A'[e,lm]*onehot_k (A' = A*s_W) -> int8 Atq with per-
  PSUM-row scale s_col[t,lm,k], folded into the host output gather.

Device per tile (128 edges, <=8 nodes):
  u|at = cast(int8 payload [256 W | 128 At]) -> bf16     (DVE, 2x)
  phase A: ps[0:96, :]   = At[:,0:96]^T  @ u[:,128:256]  (PE: l2,l3)
  phase B: ps[96:128, :] = At[:,96:128]^T @ u[:,0:128]   (PE: l0,l1)
  extraction (ACT) -> bf16 stage -> 4-fragment DMA to slots[row,t,c]
Host: slots * s_col -> scatter-add per tile -> * node_feats.
"""

import sys

sys.path.insert(0, "/opt/trn_rl_repo")

import numpy as np
import ml_dtypes

import concourse.bass as bass
import concourse.bacc as bacc
import concourse.tile as tile
from concourse import mybir
from concourse.bass_utils import run_bass_kernel_spmd

NPBF = ml_dtypes.bfloat16
BF16 = mybir.dt.bfloat16
I8 = mybir.dt.int8
F32 = mybir.dt.float32

NNODES = 25000
NEDGES = 400000
NCHAN = 64
N_CORES = 8
NPC = NNODES // N_CORES        # nodes per core
TB = 384                       # int8 bytes per tile per partition
CHUNK = 32                     # tiles per chunk (stage + out-DMA granularity)
HC = 16                        # tiles per in-DMA + cast op
PSB = 8                        # tiles per PSUM batch

L_OF_LM = np.array([0, 1, 1, 1, 2, 2, 2, 2, 2, 3, 3, 3, 3, 3, 3, 3], np.int64)
# row-block order of lm in At / PSUM / slots: l2,l3 first (96 rows), then
# l0,l1 (32 rows) - phase A covers rows 0:96, phase B rows 96:128.
PERM_LM = np.array([4, 5, 6, 7, 8, 9, 10, 11, 12, 13, 14, 15, 0, 1, 2, 3])
L_OF_PB = L_OF_LM[PERM_LM]     # l value per perm-block index

_PROGRAM_CACHE = {}


def _greedy_groups(deg, node0):
    """Group consecutive nodes: <=8 nodes, <=128 edges per group."""
    groups = []
    n = len(deg)
    i = 0
    while i < n:
        if deg[i] > 128:
            rem = deg[i]
            while rem > 0:
                take = min(128, rem)
                groups.append((node0 + i, 1, take))
                rem -= take
            i += 1
            continue
        cnt = 0
        edges = 0
        while i + cnt < n and cnt < 8 and edges + deg[i + cnt] <= 128:
            edges += deg[i + cnt]
            cnt += 1
        groups.append((node0 + i, cnt, edges))
        i += cnt
    return groups


def _build_schedule(receiver_list):
    recv = np.asarray(receiver_list).astype(np.int64)
    deg = np.bincount(recv, minlength=NNODES)
    per_core = []
    for c in range(N_CORES):
        per_core.append(_greedy_groups(deg[c * NPC:(c + 1) * NPC], c * NPC))
    t_max = max(len(g) for g in per_core)
    t_u = -(-t_max // PSB) * PSB  # round up to PSUM batch
    return recv, deg, per_core, t_u


def _pack_inputs(edge_attrs, tp_weights, recv, per_core, t_u):
    a_f = np.asarray(edge_attrs, np.float32)
    w_f = np.asarray(tp_weights, np.float32).reshape(NEDGES, 4, NCHAN)
    node_e0 = np.searchsorted(recv, np.arange(NNODES + 1))

    # --- global edge quantization (vectorized over all edges) ---
    s_w = np.abs(w_f).max(axis=2) / 127.0            # [E, 4]
    s_w = np.maximum(s_w, 1e-30)
    wq = np.rint(w_f / s_w[:, :, None]).astype(np.int8).reshape(NEDGES, 256)
    a_p = a_f[:, PERM_LM] * s_w[:, L_OF_PB]          # A' [E, 16] perm order

    in_maps = []
    slot_maps = []   # per core: (node_start, n_nodes) per tile
    scale_maps = []  # per core: s_col [T, 16, 8] fp32
    for c in range(N_CORES):
        groups = per_core[c]
        T = t_u
        # per-edge tile assignment for this core
        n_e = sum(g[2] for g in groups)
        pos = np.empty(n_e, np.int64)     # row in [T*128] canvas
        tk = np.empty(n_e, np.int64)      # t*8 + loc (column-group id)
        eidx = np.empty(n_e, np.int64)    # global edge index
        smap = []
        e_cursor = {}
        o = 0
        for t, (n0, k, ne) in enumerate(groups):
            smap.append((n0, k))
            if ne == 0:
                continue
            e0 = node_e0[n0] + e_cursor.get(n0, 0) if k == 1 else node_e0[n0]
            if k == 1:
                e_cursor[n0] = e_cursor.get(n0, 0) + ne
            sl = slice(o, o + ne)
            eidx[sl] = np.arange(e0, e0 + ne)
            pos[sl] = t * 128 + np.arange(ne)
            loc = (recv[e0:e0 + ne] - n0) if k > 1 else np.zeros(ne, np.int64)
            tk[sl] = t * 8 + loc
            o += ne
        while len(smap) < T:
            smap.append((0, 0))

        # per-column scales s_col[t*8+loc, pb]
        a_core = a_p[eidx]                              # [n_e, 16]
        s_col = np.zeros((T * 8, 16), np.float32)
        np.maximum.at(s_col, tk, np.abs(a_core))
        s_col = np.maximum(s_col / 127.0, 1e-30)
        atq_vals = np.rint(a_core / s_col[tk]).astype(np.int8)   # [n_e, 16]

        X = np.zeros((T * 128, TB), np.int8)
        X[pos, 0:256] = wq[eidx]
        at_cols = 256 + (np.arange(16) * 8)[None, :] + (tk % 8)[:, None]
        X[pos[:, None], at_cols] = atq_vals

        buf = X.reshape(T, 128, TB).transpose(1, 0, 2).reshape(128, T * TB)
        in_maps.append({"inp": np.ascontiguousarray(buf)})
        slot_maps.append(smap)
        scale_maps.append(s_col.reshape(T, 8, 16).transpose(0, 2, 1))
    return in_maps, slot_maps, scale_maps


def _build_program(t_u):
    nc = bacc.Bacc("TRN2", target_bir_lowering=False, debug=False,
                   num_devices=N_CORES)
    T = t_u
    in_d = nc.dram_tensor("inp", [128, T * TB], I8, kind="ExternalInput").ap()
    # slots[row = perm-lm-block*8 + k, tile, c]
    out_d = nc.dram_tensor("out", [128, T, 64], BF16,
                           kind="ExternalOutput").ap()

    with tile.TileContext(nc) as tc:
        with tc.tile_pool(name="ld", bufs=8) as ld_pool, \
             tc.tile_pool(name="u", bufs=6) as u_pool, \
             tc.tile_pool(name="st", bufs=4) as st_pool, \
             tc.tile_pool(name="ps", bufs=2, space="PSUM") as ps_pool:
            hci = 0
            t0 = 0
            while t0 < T:
                ct = min(CHUNK, T - t0)
                stage = st_pool.tile([128, 2, ct, 64], BF16, tag="stage")
                h0 = 0
                while h0 < ct:
                    hc = min(HC, ct - h0)
                    th = t0 + h0
                    ld = ld_pool.tile([128, hc * TB], I8, tag="ld")
                    nc.sync.dma_start(
                        out=ld,
                        in_=bass.AP(
                            tensor=in_d.tensor, offset=th * TB,
                            ap=[[T * TB, 128], [1, hc * TB]]),
                    )
                    u = u_pool.tile([128, hc, TB], BF16, tag="u")
                    # spill ~1/7 of the cast work to the ACT engine
                    ldv = ld.rearrange("p (t b) -> p t b", b=TB)
                    if hci % 7 == 3:
                        nc.scalar.copy(u, ldv)
                    else:
                        nc.vector.tensor_copy(u, ldv)
                    hci += 1
                    ps = ps_pool.tile([128, hc, 128], F32, tag="ps")
                    for k in range(hc):
                        nc.tensor.matmul(
                            ps[0:96, k], u[:, k, 256:352],
                            u[:, k, 128:256],
                            start=True, stop=True)
                    for k in range(hc):
                        nc.tensor.matmul(
                            ps[96:128, k], u[:, k, 352:384],
                            u[:, k, 0:128],
                            start=True, stop=True,
                            tile_position=(0, 96))
                    nc.scalar.copy(
                        bass.AP(
                            tensor=stage.tensor,
                            offset=stage.offset + h0 * 64,
                            ap=[stage.ap[0], [64, hc], [ct * 64, 2],
                                [1, 64]]),
                        ps,
                    )
                    h0 += hc
                # 4 out-DMA fragments per chunk (issued from the scalar
                # queue so they can't head-of-line-block input DMAs)
                for (r0, r1, half) in ((0, 40, 0), (40, 96, 1),
                                       (96, 104, 0), (104, 128, 1)):
                    nc.scalar.dma_start(
                        out=bass.AP(
                            tensor=out_d.tensor,
                            offset=r0 * (T * 64) + t0 * 64,
                            ap=[[T * 64, r1 - r0], [64, ct], [1, 64]]),
                        in_=stage[r0:r1, half],
                    )
                t0 += ct
    nc.compile()
    return nc


def kernel(node_feats, edge_attrs, tp_weights, receiver_list, nnodes,
           _trace=False):
    node_feats = np.asarray(node_feats)
    edge_attrs = np.asarray(edge_attrs)
    tp_weights = np.asarray(tp_weights)
    receiver_list = np.asarray(receiver_list)
    nnodes = int(nnodes)
    assert node_feats.shape == (NNODES, NCHAN) and nnodes == NNODES
    assert tp_weights.shape == (NEDGES, 4, NCHAN)

    recv, deg, per_core, t_u = _build_schedule(receiver_list)
    key = int(t_u)
    if key not in _PROGRAM_CACHE:
        _PROGRAM_CACHE[key] = _build_program(t_u)
    nc = _PROGRAM_CACHE[key]

    in_maps, slot_maps, scale_maps = _pack_inputs(
        edge_attrs, tp_weights, recv, per_core, t_u)
    res = run_bass_kernel_spmd(nc, in_maps, list(range(N_CORES)),
                               trace=_trace)

    inv = np.argsort(PERM_LM)  # lm -> row-block index
    g = np.zeros((NNODES, 16, NCHAN), np.float32)
    for c in range(N_CORES):
        slots = res.results[c]["out"].astype(np.float32)  # [128, T, 64]
        slots = slots.reshape(16, 8, -1, NCHAN)           # [pb, k, T, c]
        slots *= scale_maps[c].transpose(1, 2, 0)[:, :, :, None]
        slots = slots[inv]                                # [lm, k, T, c]
        smap = slot_maps[c]
        for t, (n0, k) in enumerate(smap):
            if k == 0:
                continue
            g[n0:n0 + k] += slots[:, 0:k, t, :].transpose(1, 0, 2)
    out = g * node_feats.astype(np.float32)[:, None, :]
    if _trace:
        return out, res
    return out


# revision 8
# speedup vs baseline: 2.0126x; 1.0150x over previous
"""Trainium2 Bass kernel for InvariantMessagePassingTP (v2: int8 payload).

out[n, lm, c] = sum_{e: recv[e]=n} edge_attrs[e,lm] * tp_weights[e,l(lm),c]
                * node_feats[recv[e], c]

Key algebra: the gather index equals the scatter index, so node_feats
factors out of the segment sum:
    out[n] = node_feats[n] * G[n],  G[n,lm,c] = sum_e A[e,lm] * W[e,l,c]
The device computes G via the one-hot matmul trick; the host applies
node_feats at the end.

Quantization (all scales cancel via host-side folding):
  W[e,l,c]  -> int8 Wq with per-(e,l) scale s_W; s_W is folded into A.
  At[e,(lm,k)] = A'[e,lm]*onehot_k (A' = A*s_W) -> int8 Atq with per-
  PSUM-row scale s_col[t,lm,k], folded into the host output gather.

Device per tile (128 edges, <=8 nodes):
  u|at = cast(int8 payload [256 W | 128 At]) -> bf16     (DVE, 2x)
  phase A: ps[0:96, :]   = At[:,0:96]^T  @ u[:,128:256]  (PE: l2,l3)
  phase B: ps[96:128, :] = At[:,96:128]^T @ u[:,0:128]   (PE: l0,l1)
  extraction (ACT) -> bf16 stage -> 4-fragment DMA to slots[row,t,c]
Host: slots * s_col -> scatter-add per tile -> * node_feats.
"""

import sys

sys.path.insert(0, "/opt/trn_rl_repo")

import numpy as np
import ml_dtypes

import concourse.bass as bass
import concourse.bacc as bacc
import concourse.tile as tile
from concourse import mybir
from concourse.bass_utils import run_bass_kernel_spmd

NPBF = ml_dtypes.bfloat16
BF16 = mybir.dt.bfloat16
I8 = mybir.dt.int8
F32 = mybir.dt.float32

NNODES = 25000
NEDGES = 400000
NCHAN = 64
N_CORES = 8
NPC = NNODES // N_CORES        # nodes per core
TB = 384                       # int8 bytes per tile per partition
CHUNK = 32                     # tiles per chunk (stage + out-DMA granularity)
HC = 16                        # tiles per in-DMA + cast op
PSB = 8                        # tiles per PSUM batch

L_OF_LM = np.array([0, 1, 1, 1, 2, 2, 2, 2, 2, 3, 3, 3, 3, 3, 3, 3], np.int64)
# row-block order of lm in At / PSUM / slots: l2,l3 first (96 rows), then
# l0,l1 (32 rows) - phase A covers rows 0:96, phase B rows 96:128.
PERM_LM = np.array([4, 5, 6, 7, 8, 9, 10, 11, 12, 13, 14, 15, 0, 1, 2, 3])
L_OF_PB = L_OF_LM[PERM_LM]     # l value per perm-block index

_PROGRAM_CACHE = {}


def _greedy_groups(deg, node0):
    """Group consecutive nodes: <=8 nodes, <=128 edges per group."""
    groups = []
    n = len(deg)
    i = 0
    while i < n:
        if deg[i] > 128:
            rem = deg[i]
            while rem > 0:
                take = min(128, rem)
                groups.append((node0 + i, 1, take))
                rem -= take
            i += 1
            continue
        cnt = 0
        edges = 0
        while i + cnt < n and cnt < 8 and edges + deg[i + cnt] <= 128:
            edges += deg[i + cnt]
            cnt += 1
        groups.append((node0 + i, cnt, edges))
        i += cnt
    return groups


def _build_schedule(receiver_list):
    recv = np.asarray(receiver_list).astype(np.int64)
    deg = np.bincount(recv, minlength=NNODES)
    per_core = []
    for c in range(N_CORES):
        per_core.append(_greedy_groups(deg[c * NPC:(c + 1) * NPC], c * NPC))
    t_max = max(len(g) for g in per_core)
    t_u = -(-t_max // PSB) * PSB  # round up to PSUM batch
    return recv, deg, per_core, t_u


def _pack_inputs(edge_attrs, tp_weights, recv, per_core, t_u):
    a_f = np.asarray(edge_attrs, np.float32)
    w_f = np.asarray(tp_weights, np.float32).reshape(NEDGES, 4, NCHAN)
    node_e0 = np.searchsorted(recv, np.arange(NNODES + 1))

    # --- global edge quantization (vectorized over all edges) ---
    s_w = np.abs(w_f).max(axis=2) / 127.0            # [E, 4]
    s_w = np.maximum(s_w, 1e-30)
    wq = np.rint(w_f / s_w[:, :, None]).astype(np.int8).reshape(NEDGES, 256)
    a_p = a_f[:, PERM_LM] * s_w[:, L_OF_PB]          # A' [E, 16] perm order

    in_maps = []
    slot_maps = []   # per core: (node_start, n_nodes) per tile
    scale_maps = []  # per core: s_col [T, 16, 8] fp32
    for c in range(N_CORES):
        groups = per_core[c]
        T = t_u
        # per-edge tile assignment for this core
        n_e = sum(g[2] for g in groups)
        pos = np.empty(n_e, np.int64)     # row in [T*128] canvas
        tk = np.empty(n_e, np.int64)      # t*8 + loc (column-group id)
        eidx = np.empty(n_e, np.int64)    # global edge index
        smap = []
        e_cursor = {}
        o = 0
        for t, (n0, k, ne) in enumerate(groups):
            smap.append((n0, k))
            if ne == 0:
                continue
            e0 = node_e0[n0] + e_cursor.get(n0, 0) if k == 1 else node_e0[n0]
            if k == 1:
                e_cursor[n0] = e_cursor.get(n0, 0) + ne
            sl = slice(o, o + ne)
            eidx[sl] = np.arange(e0, e0 + ne)
            pos[sl] = t * 128 + np.arange(ne)
            loc = (recv[e0:e0 + ne] - n0) if k > 1 else np.zeros(ne, np.int64)
            tk[sl] = t * 8 + loc
            o += ne
        while len(smap) < T:
            smap.append((0, 0))

        # per-column scales s_col[t*8+loc, pb]
        a_core = a_p[eidx]                              # [n_e, 16]
        s_col = np.zeros((T * 8, 16), np.float32)
        np.maximum.at(s_col, tk, np.abs(a_core))
        s_col = np.maximum(s_col / 127.0, 1e-30)
        atq_vals = np.rint(a_core / s_col[tk]).astype(np.int8)   # [n_e, 16]

        X = np.zeros((T * 128, TB), np.int8)
        X[pos, 0:256] = wq[eidx]
        at_cols = 256 + (np.arange(16) * 8)[None, :] + (tk % 8)[:, None]
        X[pos[:, None], at_cols] = atq_vals

        buf = X.reshape(T, 128, TB).transpose(1, 0, 2).reshape(128, T * TB)
        in_maps.append({"inp": np.ascontiguousarray(buf)})
        slot_maps.append(smap)
        scale_maps.append(s_col.reshape(T, 8, 16).transpose(0, 2, 1))
    return in_maps, slot_maps, scale_maps


def _build_program(t_u):
    nc = bacc.Bacc("TRN2", target_bir_lowering=False, debug=False,
                   num_devices=N_CORES)
    T = t_u
    in_d = nc.dram_tensor("inp", [128, T * TB], I8, kind="ExternalInput").ap()
    # slots[row = perm-lm-block*8 + k, tile, c]
    out_d = nc.dram_tensor("out", [128, T, 64], BF16,
                           kind="ExternalOutput").ap()

    # chunk plan: [(t0, ct)], half-chunks [(th, hc)] per chunk
    chunks = []
    t0 = 0
    while t0 < T:
        ct = min(CHUNK, T - t0)
        hcs = []
        h0 = 0
        while h0 < ct:
            hc = min(HC, ct - h0)
            hcs.append((h0, hc))
            h0 += hc
        chunks.append((t0, ct, hcs))
        t0 += ct
    PREF = 3  # chunks of input prefetch ahead of compute

    with tile.TileContext(nc) as tc:
        with tc.tile_pool(name="ld", bufs=2 * (PREF + 2)) as ld_pool, \
             tc.tile_pool(name="u", bufs=6) as u_pool, \
             tc.tile_pool(name="st", bufs=4) as st_pool, \
             tc.tile_pool(name="ps", bufs=2, space="PSUM") as ps_pool:

            lds = {}  # (chunk_idx, h0) -> ld tile

            def issue_in_dma(ci):
                t0c, _, hcs = chunks[ci]
                for h0, hc in hcs:
                    ld = ld_pool.tile([128, hc * TB], I8, tag="ld")
                    nc.sync.dma_start(
                        out=ld,
                        in_=bass.AP(
                            tensor=in_d.tensor, offset=(t0c + h0) * TB,
                            ap=[[T * TB, 128], [1, hc * TB]]),
                    )
                    lds[(ci, h0)] = ld

            for ci in range(min(PREF, len(chunks))):
                issue_in_dma(ci)

            hci = 0
            for ci, (t0c, ct, hcs) in enumerate(chunks):
                stage = st_pool.tile([128, 2, ct, 64], BF16, tag="stage")
                for h0, hc in hcs:
                    ld = lds.pop((ci, h0))
                    u = u_pool.tile([128, hc, TB], BF16, tag="u")
                    # spill ~1/7 of the cast work to the ACT engine
                    ldv = ld.rearrange("p (t b) -> p t b", b=TB)
                    if hci % 7 == 3:
                        nc.scalar.copy(u, ldv)
                    else:
                        nc.vector.tensor_copy(u, ldv)
                    hci += 1
                    ps = ps_pool.tile([128, hc, 128], F32, tag="ps")
                    for k in range(hc):
                        nc.tensor.matmul(
                            ps[0:96, k], u[:, k, 256:352],
                            u[:, k, 128:256],
                            start=True, stop=True)
                    for k in range(hc):
                        nc.tensor.matmul(
                            ps[96:128, k], u[:, k, 352:384],
                            u[:, k, 0:128],
                            start=True, stop=True,
                            tile_position=(0, 96))
                    nc.scalar.copy(
                        bass.AP(
                            tensor=stage.tensor,
                            offset=stage.offset + h0 * 64,
                            ap=[stage.ap[0], [64, hc], [ct * 64, 2],
                                [1, 64]]),
                        ps,
                    )
                # 4 out-DMA fragments per chunk; the input DMAs run
                # PREF chunks ahead so these can't starve the pipeline
                # from the head of the sync queue.
                for (r0, r1, half) in ((0, 40, 0), (40, 96, 1),
                                       (96, 104, 0), (104, 128, 1)):
                    nc.sync.dma_start(
                        out=bass.AP(
                            tensor=out_d.tensor,
                            offset=r0 * (T * 64) + t0c * 64,
                            ap=[[T * 64, r1 - r0], [64, ct], [1, 64]]),
                        in_=stage[r0:r1, half],
                    )
                if ci + PREF < len(chunks):
                    issue_in_dma(ci + PREF)
    nc.compile()
    return nc


def kernel(node_feats, edge_attrs, tp_weights, receiver_list, nnodes,
           _trace=False):
    node_feats = np.asarray(node_feats)
    edge_attrs = np.asarray(edge_attrs)
    tp_weights = np.asarray(tp_weights)
    receiver_list = np.asarray(receiver_list)
    nnodes = int(nnodes)
    assert node_feats.shape == (NNODES, NCHAN) and nnodes == NNODES
    assert tp_weights.shape == (NEDGES, 4, NCHAN)

    recv, deg, per_core, t_u = _build_schedule(receiver_list)
    key = int(t_u)
    if key not in _PROGRAM_CACHE:
        _PROGRAM_CACHE[key] = _build_program(t_u)
    nc = _PROGRAM_CACHE[key]

    in_maps, slot_maps, scale_maps = _pack_inputs(
        edge_attrs, tp_weights, recv, per_core, t_u)
    res = run_bass_kernel_spmd(nc, in_maps, list(range(N_CORES)),
                               trace=_trace)

    inv = np.argsort(PERM_LM)  # lm -> row-block index
    g = np.zeros((NNODES, 16, NCHAN), np.float32)
    for c in range(N_CORES):
        slots = res.results[c]["out"].astype(np.float32)  # [128, T, 64]
        slots = slots.reshape(16, 8, -1, NCHAN)           # [pb, k, T, c]
        slots *= scale_maps[c].transpose(1, 2, 0)[:, :, :, None]
        slots = slots[inv]                                # [lm, k, T, c]
        smap = slot_maps[c]
        for t, (n0, k) in enumerate(smap):
            if k == 0:
                continue
            g[n0:n0 + k] += slots[:, 0:k, t, :].transpose(1, 0, 2)
    out = g * node_feats.astype(np.float32)[:, None, :]
    if _trace:
        return out, res
    return out
